# revision 1
# baseline (speedup 1.0000x reference)
"""BatchAllTripletLoss v3: label-sorted padded layout, 8-core SPMD.

Each core owns 4 classes = 128 virtual anchors (32 padded slots each).
Phase A computes pd rows [128 anchors x 512 real pts] in ONE PSUM
accumulation of six fp32r matmuls (gram + column sums of squares +
anchor-sq spread via all-(-1) weights on half-squares), then a single
Sqrt activation with scale=-2 and an eps bias. The same-class +BIG k-mask
is a host constant added on DVE. ap' values (pd[anchor, positive]) come
from a small local gram over the core's padded slots (fixed block
positions), gathered into per-group columns by 4 strided DMAs and masked
with host constants.

Phase B per 4-anchor group: one fp32r one-hot matmul replicates bias
rows into [128,512] PSUM; relu+sum runs on ACT (scale=-1, bias=ap'+m,
fused accum) for some groups and on DVE (min(b4-s,0), fused accum,
negated on host) for the rest; counting is a 4x-mode bf16 DVE pass.
Selectors are built on idle gpsimd via per-group affine_select.
"""
import os
import sys

for _p in ("/opt/trn_rl_repo",):
    if os.path.isdir(_p) and _p not in sys.path:
        sys.path.insert(0, _p)

import numpy as np

import concourse.bacc as bacc
import concourse.tile as tile
from concourse import mybir
from concourse import bass_utils

N = 512
D = 256
N_CORES = 8
W = 32                      # padded slots per class
NC_CLS = 4                  # classes per core
VA = W * NC_CLS             # 128 virtual anchors per core
NG = VA // 4                # 32 groups of 4 anchors
MARGIN = 0.1
EPS = 1e-16
D2_EPS = 0.05               # dominates fp32r rounding noise at d2~0
KMASK = 16384.0             # added to bias at same-class k columns
JMASK = -65536.0            # ap' value at invalid (anchor, j) pairs
HALF = 0.70710678118654752  # sqrt(1/2): Square(x*HALF) = x^2/2

F32 = mybir.dt.float32
F32R = mybir.dt.float32r
BF16 = mybir.dt.bfloat16
BF16_NP = mybir.dt.np(mybir.dt.bfloat16)
AF = mybir.ActivationFunctionType
OP = mybir.AluOpType

_PROGRAM_CACHE = {}


def engine_split(n):
    """Per-group relu engine: ~10/19 on ACT, rest on DVE, interleaved."""
    n_act = max(1, round(n * 12 / 19))
    acts = set()
    for i in range(n):
        if (i * n_act) // n != ((i + 1) * n_act) // n:
            acts.add(i)
    return ["act" if i in acts else "dve" for i in range(n)]


def build_program(n_rep=1, loop=None, glist=None):
    """loop=None: single-shot. loop="B": For_i around phase B.
    loop="A": For_i around input DMAs + phase A. loop="C": phase A only.
    glist: phase-B group ids (g = 8*cc + gl)."""
    if glist is None:
        glist = tuple(range(NG))
    engs = engine_split(len(glist))
    nc = bacc.Bacc(trn_type="TRN2")

    eva_d = nc.dram_tensor("embT_vanch", [128, 2, VA], F32R, kind="ExternalInput")
    etr_d = nc.dram_tensor("ET_real", [128, 2, N], F32R, kind="ExternalInput")
    mskb_d = nc.dram_tensor("maskbig", [128, N], BF16, kind="ExternalInput")
    blob_d = nc.dram_tensor("blob", [128, 704], F32R, kind="ExternalInput")
    rsum_d = nc.dram_tensor("rsum", [128, NG], F32, kind="ExternalOutput")
    csum_d = nc.dram_tensor("csum", [128, NG], F32, kind="ExternalOutput")

    with tile.TileContext(nc) as tc:
        with tc.tile_pool(name="persist", bufs=1) as persist, \
             tc.tile_pool(name="psA", bufs=2, space="PSUM") as psA, \
             tc.tile_pool(name="psumB", bufs=3, space="PSUM") as psumB, \
             tc.tile_pool(name="rB", bufs=6) as rB, \
             tc.tile_pool(name="gB", bufs=3) as gB:

            eva_sb = persist.tile([128, 2, VA], F32R)
            etr_sb = persist.tile([128, 2, N], F32R)
            mskb_sb = persist.tile([128, N], BF16)
            blob_sb = persist.tile([128, 704], F32R)
            ones_sb = blob_sb[:, 0:128]          # 1.0
            apM_sb = blob_sb[:, 128:160].bitcast(F32)   # ap' mask
            apC_sb = blob_sb[:, 160:192].bitcast(F32)   # ap' offset
            negones_sb = blob_sb[:, 192:704]     # -1.0
            sq2_sb = persist.tile([128, 2, N], F32R)
            sqeva_sb = persist.tile([128, 2, VA], F32R)
            pd_sb = persist.tile([128, N], F32)
            pdl_sb = persist.tile([128, VA], F32)
            bias_sb = persist.tile([128, N], F32R)
            sel_sb = persist.tile([128, len(glist) * 128], F32R)
            eps_sb = persist.tile([128, 1], F32)
            dum_sb = persist.tile([1, 1], F32)
            apraw_sb = persist.tile([128, NG], F32)
            apcol_sb = persist.tile([128, NG], F32)
            out_sb = persist.tile([128, 2 * NG], F32)
            rsum_sb = out_sb[:, 0:NG]
            csum_sb = out_sb[:, NG:2 * NG]

            def setup():
                nc.sync.dma_start(blob_sb[:], blob_d.ap()[:])
                nc.vector.memset(eps_sb[:], D2_EPS)
                nc.vector.memset(out_sb[:], 0.0)
                # pin the sqrt_and_others ACT table once, off-critical-path
                nc.scalar.activation(dum_sb[:], eps_sb[0:1, :], AF.Sqrt)
                # selectors + phase-B one-hots on idle gpsimd
                for i, g in enumerate(glist):
                    o4 = sel_sb[:, i * 128:(i + 1) * 128].rearrange(
                        "p (w q) -> p w q", q=4)
                    i4 = ones_sb.rearrange("p (w q) -> p w q", q=4)
                    nc.gpsimd.affine_select(
                        o4, i4, [[0, W], [-1, 4]], OP.is_equal, 0.0,
                        base=-4 * g, channel_multiplier=1)

            def input_dmas():
                nc.sync.dma_start(eva_sb[:], eva_d.ap()[:])
                nc.sync.dma_start(etr_sb[:], etr_d.ap()[:])
                nc.sync.dma_start(mskb_sb[:], mskb_d.ap()[:])

            def phase_a():
                # half-squares for the rank-128 sq spreads
                nc.scalar.activation(sqeva_sb[:], eva_sb[:], AF.Square,
                                     scale=HALF)
                # local psum = gl - sq_a/2 - sq_a'/2 over the 128 slots
                dl_ps = psA.tile([VA, VA], F32, tag="dl")
                for h in range(2):
                    nc.tensor.matmul(dl_ps[:],
                                     lhsT=eva_sb[:, h, :],
                                     rhs=eva_sb[:, h, :],
                                     start=(h == 0), stop=False)
                for h in range(2):
                    nc.tensor.matmul(dl_ps[:],
                                     lhsT=sqeva_sb[:, h, :],
                                     rhs=negones_sb[:, 0:VA],
                                     start=False, stop=False)
                for h in range(2):
                    nc.tensor.matmul(dl_ps[:],
                                     lhsT=negones_sb[:, 0:VA],
                                     rhs=sqeva_sb[:, h, :],
                                     start=False, stop=(h == 1))
                nc.scalar.activation(pdl_sb[:], dl_ps[:], AF.Sqrt,
                                     bias=eps_sb[:], scale=-2.0)
                nc.scalar.activation(sq2_sb[:], etr_sb[:], AF.Square,
                                     scale=HALF)
                # ap' gather: apraw[4w+q, 8cc+gl] = pdl[32cc+w, 32cc+4gl+q];
                # mask per class block so groups unblock progressively
                dstq = apraw_sb.rearrange("(w q) c -> w q c", q=4)
                for cc in range(NC_CLS):
                    ngl = 0
                    for g in glist:
                        if g // 8 == cc:
                            ngl = max(ngl, g % 8 + 1)
                    if ngl == 0:
                        continue
                    src3 = pdl_sb[32 * cc:32 * cc + 32,
                                  32 * cc:32 * cc + 4 * ngl]
                    src3 = src3.rearrange("w (gl q) -> w gl q", q=4)
                    for q in range(4):
                        nc.sync.dma_start(
                            dstq[:, q:q + 1, 8 * cc:8 * cc + ngl].squeeze(),
                            src3[:, :, q:q + 1].squeeze())
                    sl = slice(8 * cc, 8 * cc + ngl)
                    nc.vector.tensor_tensor(apcol_sb[:, sl], apraw_sb[:, sl],
                                            apM_sb[:, sl], op=OP.mult)
                    nc.vector.tensor_tensor(apcol_sb[:, sl], apcol_sb[:, sl],
                                            apC_sb[:, sl], op=OP.add)

                # real-k psum = g - sq_a/2 - sq_k/2   (fp32r, free=512)
                d2_ps = psA.tile([VA, N], F32, tag="d2")
                for h in range(2):
                    nc.tensor.matmul(d2_ps[:],
                                     lhsT=eva_sb[:, h, :],
                                     rhs=etr_sb[:, h, :],
                                     start=(h == 0), stop=False)
                for h in range(2):
                    nc.tensor.matmul(d2_ps[:],
                                     lhsT=sqeva_sb[:, h, :],
                                     rhs=negones_sb,
                                     start=False, stop=False)
                for h in range(2):
                    nc.tensor.matmul(d2_ps[:],
                                     lhsT=negones_sb[:, 0:VA],
                                     rhs=sq2_sb[:, h, :],
                                     start=False, stop=(h == 1))
                # pd = sqrt(-2*psum + eps)
                nc.scalar.activation(pd_sb[:], d2_ps[:], AF.Sqrt,
                                     bias=eps_sb[:], scale=-2.0)
                # bias rows: pd + KMASK at same-class columns
                nc.vector.tensor_tensor(bias_sb[:], pd_sb[:], mskb_sb[:],
                                        op=OP.add)

            def phase_b():
                for i, g in enumerate(glist):
                    b4 = psumB.tile([128, N], F32, tag="b4")
                    nc.tensor.matmul(
                        b4[:],
                        lhsT=sel_sb[:, i * 128:(i + 1) * 128],
                        rhs=bias_sb[:],
                        start=True, stop=True)
                    if engs[i] == "act":
                        R = rB.tile([128, N], BF16, tag="R")
                        nc.scalar.activation(
                            R[:], b4[:], AF.Relu,
                            bias=apcol_sb[:, g:g + 1], scale=-1.0,
                            accum_out=rsum_sb[:, g:g + 1])
                        G = gB.tile([128, N], BF16, tag="G")
                        nc.vector.tensor_scalar(
                            G[:], R[:], 0.0, None, op0=OP.is_gt, op1=OP.add,
                            accum_out=csum_sb[:, g:g + 1])
                    else:
                        # R2 = min(b4-s, 0) = -relu(s-b4); DVE sums, GPS counts
                        R = rB.tile([128, N], BF16, tag="R")
                        nc.vector.tensor_scalar(
                            R[:], b4[:], apcol_sb[:, g:g + 1], 0.0,
                            op0=OP.subtract, op1=OP.min)
                        G2 = gB.tile([128, N], BF16, tag="G2")
                        nc.vector.tensor_scalar(
                            G2[:], R[:], 0.0, None, op0=OP.add, op1=OP.add,
                            accum_out=rsum_sb[:, g:g + 1])
                        G = gB.tile([128, N], BF16, tag="G")
                        nc.vector.tensor_scalar(
                            G[:], R[:], 0.0, None, op0=OP.is_lt, op1=OP.add,
                            accum_out=csum_sb[:, g:g + 1])

            setup()
            if loop is None:
                input_dmas()
                phase_a()
                phase_b()
            elif loop == "B":
                input_dmas()
                phase_a()
                with tc.For_i(0, n_rep, 1):
                    phase_b()
            elif loop == "A":
                with tc.For_i(0, n_rep, 1):
                    input_dmas()
                    phase_a()
            elif loop == "C":
                input_dmas()
                with tc.For_i(0, n_rep, 1):
                    phase_a()
            else:
                raise ValueError(loop)

            nc.sync.dma_start(rsum_d.ap()[:], rsum_sb[:])
            nc.sync.dma_start(csum_d.ap()[:], csum_sb[:])

    nc.compile()
    return nc


def get_program(n_rep=1, loop=None, glist=None):
    key = (n_rep, loop, glist)
    if key not in _PROGRAM_CACHE:
        _PROGRAM_CACHE[key] = build_program(n_rep, loop, glist)
    return _PROGRAM_CACHE[key]


def host_layout(embeddings, labels):
    """Sort by label into padded virtual slots; deal classes to
    (core, position) by descending size to minimize group counts."""
    emb = np.ascontiguousarray(np.asarray(embeddings, dtype=np.float32))
    lab = np.asarray(labels).astype(np.int64)
    assert emb.shape == (N, D)
    perm = np.argsort(lab, kind="stable")
    emb_p = emb[perm]
    counts = np.bincount(lab, minlength=32)
    assert counts.max() <= W, f"class too large for W={W}: {counts.max()}"
    starts = np.zeros(33, dtype=np.int64)
    starts[1:] = np.cumsum(counts)
    order = np.argsort(-counts, kind="stable")
    glist = []
    core_cls = np.zeros((N_CORES, NC_CLS), dtype=np.int64)
    for cc in range(NC_CLS):
        chunk = order[cc * N_CORES:(cc + 1) * N_CORES]
        for gl in range((counts[chunk].max() + 3) // 4):
            glist.append(8 * cc + gl)
        core_cls[:, cc] = chunk
    return emb_p, counts, starts, core_cls, tuple(sorted(glist))


def make_in_maps(embeddings, labels):
    emb_p, counts, starts, core_cls, glist = host_layout(embeddings, labels)
    ET_real = np.ascontiguousarray(
        emb_p.T.reshape(2, 128, N).transpose(1, 0, 2))          # [128,2,512]
    in_maps = []
    for c in range(N_CORES):
        v_emb = np.zeros((VA, D), dtype=np.float32)
        mskb = np.zeros((128, N), dtype=np.float32)
        apM = np.zeros((128, NG), dtype=np.float32)
        apC = np.full((128, NG), JMASK, dtype=np.float32)
        for cc in range(NC_CLS):
            cls = core_cls[c, cc]
            m = int(counts[cls])
            s = int(starts[cls])
            if m:
                v_emb[cc * W:cc * W + m] = emb_p[s:s + m]
            # bias k-mask for this block's 32 anchor rows
            mskb[cc * W:cc * W + 32, s:s + m] = KMASK
            # ap' validity: partition p = 32q+w -> anchor 4gl+q of block,
            # j slot w; valid iff both < m and distinct
            for gl in range(8):
                g = 8 * cc + gl
                if g not in glist:
                    continue
                for q in range(4):
                    a = 4 * gl + q
                    if a >= m:
                        continue
                    for w in range(m):
                        if w == a:
                            continue
                        apM[4 * w + q, g] = 1.0
                        apC[4 * w + q, g] = MARGIN
        eva = np.ascontiguousarray(
            v_emb.T.reshape(2, 128, VA).transpose(1, 0, 2))
        blob = np.zeros((128, 704), dtype=np.float32)
        blob[:, 0:128] = 1.0
        blob[:, 128:160] = apM
        blob[:, 160:192] = apC
        blob[:, 192:704] = -1.0
        in_maps.append({
            "embT_vanch": eva,
            "ET_real": ET_real,
            "maskbig": mskb.astype(BF16_NP),
            "blob": blob,
        })
    return in_maps, glist


def reduce_outputs(results, glist):
    engs = engine_split(len(glist))
    loss_sum = 0.0
    hard_sum = 0.0
    for r in results:
        rs = r["rsum"].astype(np.float64)
        cs = r["csum"].astype(np.float64)
        for i, g in enumerate(glist):
            sgn = 1.0 if engs[i] == "act" else -1.0
            loss_sum += sgn * rs[:, g].sum()
            hard_sum += cs[:, g].sum()
    num_hard = np.float32(hard_sum)
    loss = np.float32(np.float32(loss_sum) / (num_hard + np.float32(EPS)))
    return loss, num_hard


def kernel(embeddings, labels):
    in_maps, glist = make_in_maps(embeddings, labels)
    nc = get_program(glist=glist)
    res = bass_utils.run_bass_kernel_spmd(
        nc, in_maps, core_ids=list(range(N_CORES)))
    return reduce_outputs(res.results, glist)



# revision 2
# speedup vs baseline: 1.0808x; 1.0808x over previous
"""BatchAllTripletLoss v4: (slot, rank) windowed layout, 8-core SPMD.

Each core's 128 anchor slots are a contiguous 128-point window of the
label-sorted embedding (the host rolls the sorted array per core so the
window sits at columns 0:128 of that core's ET).  Anchors whose class
needs more positive-ranks than T are covered by several overlapping
windows (greedy interval multicover), so every core runs the same
T = max-ranks-per-slot tiles (T=12 here, provably minimal: 8*128 slot
instances must cover sum_p ceil((m_p-1)/T) requirements).

Phase A: the pd row block [128 slots x 512 pts] comes from six fp32r
matmuls into one PSUM bank, then one ACT Sqrt to f32.  pd is recentered
by sqrt(2D) and cast to f16 (one f16 ulp ~0.002 << margin 0.1; raw
distances concentrate at ~22.6 where even bf16/f16 ulps would swamp the
margin).  bias = pdc + host K-mask (same-class columns pushed out of
relu range).  ap' values pdc[slot, positive_t] come from one gpsimd
indirect_copy (indices are shared per 16-partition group, so it gathers
a 16x16 block per tile) followed by a diagonal-extract multiply with a
host 0/1 mask and a tensor_reduce.

Phase B per tile t sweeps the SAME f16 bias tile (no replication
matmul, no PSUM).  On HW every reduction (accum_out) costs ~600-800ns
regardless of DVE perf mode, so the 2T reductions are split across
engines: most sums run on ACT as relu(ap'-bias) with fused accumulate,
the rest on DVE as a 4x-mode min pass plus a sum-accum (negated on
host); counts run on DVE as is_lt+accum, with a few on ACT via a Sign
pass (per-partition sign-sum = 2*count - 512, undone on host).

The timing builds wrap each phase in a hardware For_i whose body holds
8 copies of the phase: this amortizes the loop's all-engine barrier
(~1.7us) and lets copies pipeline through 4-way-buffered tiles.
"""
import os
import sys

for _p in ("/opt/trn_rl_repo",):
    if os.path.isdir(_p) and _p not in sys.path:
        sys.path.insert(0, _p)

import numpy as np

import concourse.bacc as bacc
import concourse.tile as tile
from concourse import mybir
from concourse import bass_utils

N = 512
D = 256
N_CORES = 8
NCLS = 32
MARGIN = 0.1
EPS = 1e-16
D2_EPS = 0.05               # dominates fp32r rounding noise at d2~0
KMASK = 16384.0             # added to bias at same-class columns
ABIG = -65536.0             # apcol value for invalid (slot, rank)
HALF = 0.70710678118654752  # sqrt(1/2): Square(x*HALF) = x^2/2
CENTER = 22.62741699796952    # sqrt(2D): distances concentrate here; pd is
                              # recentered before the f16 cast so one f16 ulp
                              # (~0.002) stays far below the 0.1 margin

F32 = mybir.dt.float32
F32R = mybir.dt.float32r
BF16 = mybir.dt.bfloat16
U16 = mybir.dt.uint16
F16 = mybir.dt.float16
BF16_NP = mybir.dt.np(mybir.dt.bfloat16)
F16_NP = mybir.dt.np(mybir.dt.float16)
AF = mybir.ActivationFunctionType
OP = mybir.AluOpType
AX = mybir.AxisListType

_PROGRAM_CACHE = {}


def spread(T, n, offset=0):
    """n tile indices spread evenly over range(T), rotated by offset."""
    if n <= 0:
        return set()
    return {(offset + (i * T) // n) % T for i in range(n)}


def engine_split(T):
    """Per-tile engine for (sum, count).  HW reduction ops (accum_out)
    cost ~600-800ns regardless of dtype mode, so balance the 2T
    reductions across ACT and DVE: ACT takes ~3/4 of the sums
    (relu+accum) and a few counts (Sign+accum); DVE takes the rest
    (min+sum pair / is_lt)."""
    n_as = round(T * 0.83)
    n_ac = max(0, round(T * 0.17))
    acts_s = spread(T, n_as)
    acts_c = spread(T, n_ac, offset=T // 2)
    return (["act" if i in acts_s else "dve" for i in range(T)],
            ["act" if i in acts_c else "dve" for i in range(T)])


def blob_layout(T):
    T2 = (T + 1) // 2
    c_apM = T2
    c_apC = T2 + T
    c_E16 = T2 + 2 * T
    c_eps = c_E16 + 8 * T
    B = c_eps + 1
    return T2, c_apM, c_apC, c_E16, c_eps, B


def build_program(T, n_rep=1, loop=None, unroll=1):
    """loop=None: single-shot. loop="B": For_i around phase B.
    loop="A": For_i around input DMAs + phase A.  The loop body holds
    `unroll` copies of the phase (amortizes the For_i all-engine
    barrier and lets consecutive copies pipeline); n_rep counts phase
    executions and must be divisible by unroll."""
    engs_s, engs_c = engine_split(T)
    T2, c_apM, c_apC, c_E16, c_eps, B = blob_layout(T)
    assert n_rep % unroll == 0
    nc = bacc.Bacc(trn_type="TRN2")

    et_d = nc.dram_tensor("ET", [128, 2, N], F32R, kind="ExternalInput")
    msk_d = nc.dram_tensor("mskb", [128, N], F16, kind="ExternalInput")
    blob_d = nc.dram_tensor("blob", [128, B], F32, kind="ExternalInput")
    out_d = nc.dram_tensor("out", [128, 2 * T], F32, kind="ExternalOutput")

    with tile.TileContext(nc) as tc:
        with tc.tile_pool(name="persist", bufs=1) as persist, \
             tc.tile_pool(name="pa", bufs=4) as pa, \
             tc.tile_pool(name="psA", bufs=4, space="PSUM") as psA, \
             tc.tile_pool(name="sc", bufs=4) as sc:

            neg_sb = persist.tile([128, N], F32)
            out_sb = persist.tile([128, 2 * T], F32)
            dum_sb = persist.tile([1, 1], F32)
            one_sb = persist.tile([1, 1], F32)

            def setup():
                nc.vector.memset(neg_sb[:], -1.0)
                nc.vector.memset(out_sb[:], 0.0)
                nc.vector.memset(one_sb[:], 1.0)
                # pin the sqrt_and_others ACT table once, off-critical-path
                nc.scalar.activation(dum_sb[:], one_sb[:], AF.Sqrt)

            def phase_a():
                """input DMAs + pd/bias/apcol prep; returns (bias, apcol).
                All tiles come from the double-buffered `pa` pool so
                consecutive copies can overlap."""
                et_sb = pa.tile([128, 2, N], F32R, tag="et")
                msk_sb = pa.tile([128, N], F16, tag="msk")
                blob_sb = pa.tile([128, B], F32, tag="blob")
                sq2_sb = pa.tile([128, 2, N], F32R, tag="sq2")
                pd_sb = pa.tile([128, N], F32, tag="pd")
                pdc_sb = pa.tile([128, N], F16, tag="pdc")
                bias_sb = pa.tile([128, N], F16, tag="bias")
                diag_sb = pa.tile([128, T, 16], F16, tag="diag")
                prod_sb = pa.tile([128, T, 16], F16, tag="prod")
                apraw_sb = pa.tile([128, T], F32, tag="apraw")
                apcol_sb = pa.tile([128, T], F32, tag="apcol")

                idx_v = blob_sb[:, 0:T2].bitcast(U16)
                apM_v = blob_sb[:, c_apM:c_apM + T]
                apC_v = blob_sb[:, c_apC:c_apC + T]
                e16_v = blob_sb[:, c_E16:c_E16 + 8 * T].bitcast(F16)
                eps_v = blob_sb[:, c_eps:c_eps + 1]

                nc.sync.dma_start(et_sb[:], et_d.ap()[:])
                nc.sync.dma_start(msk_sb[:], msk_d.ap()[:])
                nc.sync.dma_start(blob_sb[:], blob_d.ap()[:])

                nc.scalar.activation(sq2_sb[:], et_sb[:], AF.Square,
                                     scale=HALF)
                d2 = psA.tile([128, N], F32, tag="d2")
                for h in range(2):
                    nc.tensor.matmul(d2[:], lhsT=et_sb[:, h, 0:128],
                                     rhs=et_sb[:, h, :],
                                     start=(h == 0), stop=False)
                for h in range(2):
                    nc.tensor.matmul(d2[:], lhsT=sq2_sb[:, h, 0:128],
                                     rhs=neg_sb.bitcast(F32R)[:],
                                     start=False, stop=False)
                for h in range(2):
                    nc.tensor.matmul(d2[:], lhsT=neg_sb.bitcast(F32R)[:, 0:128],
                                     rhs=sq2_sb[:, h, :],
                                     start=False, stop=(h == 1))
                # pd = sqrt(-2*psum + eps)
                nc.scalar.activation(pd_sb[:], d2[:], AF.Sqrt,
                                     bias=eps_v, scale=-2.0)
                # recenter so f16 keeps ~0.002 resolution near the margin
                nc.vector.tensor_scalar(pdc_sb[:], pd_sb[:], CENTER, 0.0,
                                        op0=OP.subtract, op1=OP.add)
                # ap' gather: diag[p, 16t+q] = pdc[p, idxJ[16g+q, t]]
                nc.gpsimd.indirect_copy(
                    diag_sb.rearrange("p a b -> p (a b)"),
                    pdc_sb[:], idx_v[:, 0:T], True)
                # bias rows: pd + KMASK at same-class columns
                nc.vector.tensor_tensor(bias_sb[:], pdc_sb[:], msk_sb[:],
                                        op=OP.add)
                # diagonal extract: apraw[p,t] = sum_q diag[p,16t+q]*E16[p,q]
                nc.vector.tensor_tensor(
                    prod_sb.rearrange("p a b -> p (a b)"),
                    diag_sb.rearrange("p a b -> p (a b)"),
                    e16_v, op=OP.mult)
                nc.vector.tensor_reduce(apraw_sb[:], prod_sb[:],
                                        axis=AX.X, op=OP.add)
                # apcol = apraw*apM + apC  (valid: ap'+margin, else -BIG)
                nc.vector.tensor_tensor(apraw_sb[:], apraw_sb[:], apM_v,
                                        op=OP.mult)
                nc.vector.tensor_tensor(apcol_sb[:], apraw_sb[:], apC_v,
                                        op=OP.add)
                return bias_sb, apcol_sb

            def phase_b(bias_sb, apcol_sb):
                for t in range(T):
                    ap_t = apcol_sb[:, t:t + 1]
                    if engs_s[t] == "act":
                        # sum: relu(ap - bias), fused accumulate
                        R = sc.tile([128, N], F16, tag="R")
                        nc.scalar.activation(
                            R[:], bias_sb[:], AF.Relu,
                            bias=ap_t, scale=-1.0,
                            accum_out=out_sb[:, t:t + 1])
                    else:
                        # 4x-mode min pass, then 1x sum-accum (negated sum)
                        R = sc.tile([128, N], F16, tag="R")
                        nc.vector.tensor_scalar(
                            R[:], bias_sb[:], ap_t, 0.0,
                            op0=OP.subtract, op1=OP.min)
                        S = sc.tile([128, N], F16, tag="S")
                        nc.vector.tensor_scalar(
                            S[:], R[:], 0.0, None,
                            op0=OP.add, op1=OP.add,
                            accum_out=out_sb[:, t:t + 1])
                    if engs_c[t] == "act":
                        # count via sign: sum sign(ap - bias) = 2C - 512
                        G = sc.tile([128, N], F16, tag="G")
                        nc.scalar.activation(
                            G[:], bias_sb[:], AF.Sign,
                            bias=ap_t, scale=-1.0,
                            accum_out=out_sb[:, T + t:T + t + 1])
                    else:
                        G = sc.tile([128, N], F16, tag="G")
                        nc.vector.tensor_scalar(
                            G[:], bias_sb[:], ap_t, None,
                            op0=OP.is_lt, op1=OP.add,
                            accum_out=out_sb[:, T + t:T + t + 1])

            setup()
            if loop is None:
                ba = phase_a()
                phase_b(*ba)
            elif loop == "B":
                ba = phase_a()
                with tc.For_i(0, n_rep // unroll, 1):
                    for _ in range(unroll):
                        phase_b(*ba)
            elif loop == "A":
                with tc.For_i(0, n_rep // unroll, 1):
                    for _ in range(unroll):
                        phase_a()
            else:
                raise ValueError(loop)

            nc.sync.dma_start(out_d.ap()[:], out_sb[:])

    nc.compile()
    return nc


def get_program(T, n_rep=1, loop=None):
    unroll = 8 if (loop is not None and n_rep % 8 == 0) else 1
    key = (T, n_rep, loop, unroll)
    if key not in _PROGRAM_CACHE:
        _PROGRAM_CACHE[key] = build_program(T, n_rep, loop, unroll)
    return _PROGRAM_CACHE[key]


def host_layout(labels):
    """Label-sort; pick minimal T such that 8 windows of 128 contiguous
    sorted positions can cover every anchor position P with multiplicity
    ceil((m_P - 1)/T) (each covering handles up to T positive-ranks).
    Classic greedy interval multicover: repeatedly place a window at the
    first position with unmet requirement.  Returns (perm, counts,
    starts, T, windows); windows = list of (window_start,
    {abs_position: (r0, r1)})."""
    lab = np.asarray(labels).astype(np.int64)
    counts = np.bincount(lab, minlength=NCLS)
    perm = np.argsort(lab, kind="stable")
    starts = np.zeros(NCLS + 1, dtype=np.int64)
    starts[1:] = np.cumsum(counts)
    lab_s = lab[perm]
    need = np.maximum(counts[lab_s] - 1, 0)      # ranks needed per position

    wins = []
    for T in range(1, 64):
        req = -(-need // T)                      # ceil
        covered = np.zeros(N, dtype=np.int64)
        wins = []
        ok = True
        while True:
            unmet = np.nonzero(covered < req)[0]
            if len(unmet) == 0:
                break
            p = int(unmet[0])
            wins.append(p)
            covered[p:p + 128] += 1
            if len(wins) > N_CORES:
                ok = False
                break
        if ok:
            break

    windows = []
    taken = np.zeros(N, dtype=np.int64)
    for ws in wins:
        asg = {}
        for p in range(ws, min(ws + 128, N)):
            rem = int(need[p] - taken[p])
            if rem > 0:
                take = min(T, rem)
                asg[p] = (int(taken[p]), int(taken[p]) + take)
                taken[p] += take
        windows.append((ws, asg))
    while len(windows) < N_CORES:
        windows.append((0, {}))
    return perm, counts, starts, T, windows


def make_in_maps(embeddings, labels):
    emb = np.ascontiguousarray(np.asarray(embeddings, dtype=np.float32))
    assert emb.shape == (N, D)
    perm, counts, starts, T, windows = host_layout(labels)
    emb_s = emb[perm]
    lab_s = np.asarray(labels).astype(np.int64)[perm]
    T2, c_apM, c_apC, c_E16, c_eps, B = blob_layout(T)

    e16t = np.zeros((128, 16 * T), dtype=np.float32)
    p16 = np.arange(128) % 16
    for t in range(T):
        e16t[np.arange(128), 16 * t + p16] = 1.0

    in_maps = []
    for q in range(N_CORES):
        ws, asg = windows[q]
        cols = (ws + np.arange(N)) % N
        emb_r = emb_s[cols]                       # [512, 256]
        lab_r = lab_s[cols]
        ET = np.ascontiguousarray(
            emb_r.T.reshape(2, 128, N).transpose(1, 0, 2))   # [128,2,512]
        mskb = (KMASK * (lab_r[:128, None] == lab_r[None, :])).astype(F16_NP)

        idxJ = np.zeros((128, T), dtype=np.uint16)
        apM = np.zeros((128, T), dtype=np.float32)
        apC = np.full((128, T), ABIG, dtype=np.float32)
        for p_abs, (r0, r1) in asg.items():
            slot = int(p_abs - ws)
            c = lab_s[p_abs]
            members = np.arange(starts[c], starts[c + 1])
            others = members[members != p_abs]
            for t in range(r1 - r0):
                j_abs = others[r0 + t]
                idxJ[slot, t] = (j_abs - ws) % N
                apM[slot, t] = 1.0
                apC[slot, t] = MARGIN

        blob = np.zeros((128, B), dtype=np.float32)
        blob[:, 0:T2].view(np.uint16)[:, 0:T] = idxJ
        blob[:, c_apM:c_apM + T] = apM
        blob[:, c_apC:c_apC + T] = apC
        blob[:, c_E16:c_E16 + 8 * T].view(F16_NP)[:, :] = e16t.astype(F16_NP)
        blob[:, c_eps] = D2_EPS

        in_maps.append({"ET": ET, "mskb": mskb, "blob": blob})
    return in_maps, T


def reduce_outputs(results, T):
    engs_s, engs_c = engine_split(T)
    loss_sum = 0.0
    hard_sum = 0.0
    for r in results:
        o = r["out"].astype(np.float64)
        for t in range(T):
            sgn = 1.0 if engs_s[t] == "act" else -1.0
            loss_sum += sgn * o[:, t].sum()
            if engs_c[t] == "act":
                # sign-sum per partition: 2C - 512  ->  C
                hard_sum += ((o[:, T + t] + N) / 2.0).sum()
            else:
                hard_sum += o[:, T + t].sum()
    num_hard = np.float32(hard_sum)
    loss = np.float32(np.float32(loss_sum) / (num_hard + np.float32(EPS)))
    return loss, num_hard


def kernel(embeddings, labels):
    in_maps, T = make_in_maps(embeddings, labels)
    nc = get_program(T)
    res = bass_utils.run_bass_kernel_spmd(
        nc, in_maps, core_ids=list(range(N_CORES)))
    return reduce_outputs(res.results, T)
